# revision 47
# baseline (speedup 1.0000x reference)
"""Biaffine span classifier kernel for 8 Trainium2 NeuronCores.

Math (per batch b, label o):
    start = relu(x @ W_start + b_start); end = relu(x @ W_end + b_end)
    rotate both with tiled-halves sinusoidal tables
    span[o,x,y] = startR[x,:] @ weight[o] @ endR[y,:]^T
    span = span*pad[y] - (1-pad[y])*NEG - NEG*tril(x>y)

Sharding: core c = b*2 + half handles batch b and labels [half*8, half*8+8).

The kernel is HBM-bound, so the device moves as few bytes as possible:
  * The output is written in bf16 (per-element tolerance is 2e-2; fp32r
    matmul + bf16 rounding lands well under 1e-2) and upconverted on the
    host during the gather step.
  * Every entry at or below the diagonal band, and every masked column, is
    a value computable from `mask` alone in exact fp32 (-NEG, -2*NEG, or
    -NEG*(2-pad[y]) -- |span| << 0.5*ulp(NEG) so the reference's own fp32
    adds round to exactly these). The device only computes/writes the 36
    upper-triangular 128x128 blocks per label as eight row bands
    (rows [128k,128k+128) x cols [128k,1024)); the host fills the rest
    during unsharding. Device writes drop from 32 MB to 9.4 MB per core.
  * All matmuls run fp32r (full PE rate at free-dim >= 256). x and the
    rotation tables ship as fp16 (negligible 2^-11 rounding) to halve the
    critical-path input bytes; weights are float32r DRAM so DMA lands
    them matmul-ready.

PE utilization trick: the span contraction is K=64, which would idle half
the 128-row PE array. startR/endR are produced DUPLICATED across both
partition halves (via selector matmuls whose outputs live on partitions
0-63 and 64-127), and the per-label tmp = W_o^T startR matmuls emit label
pairs (2g, 2g+1) onto rows 0-63 / 64-127 of one tile. Span matmuls for a
label pair then issue as two K=64 matmuls at tile_position (0,0) and
(64,0) -- distinct PE row groups -- so they stream concurrently (~2x).

Scheduling: inputs load on one sync-queue FIFO in critical-path order
(descriptor generation costs ~0.6us per dma_start, so host-preswizzled
layouts keep descriptors large and DMAs few); dummy matmuls warm the PE
HAM window during the load. PSUM->SBUF bf16 casts are split DVE (even
label) / ACT (odd label) with 4-deep PSUM rings and 8-deep staging so
the PE and output DMAs run ahead of the casting engines; output DMAs
split across the sync HWDGE ring (even) and gpsimd SWDGE ring (odd).
Bands 4-7 (h=1-only data) flow while prep of h=0 runs, sandwiched after
the first band group so its serial chain hides under band casts.
"""

import numpy as np

B, S, I, H, O = 4, 1024, 768, 64, 16
NCORES = 8
OH = O // 2  # 8 labels per core
NEG = 1.0e12
KT = I // 128  # 6 k-tiles over the input dim

# band xb covers rows [128xb, 128xb+128) x cols [128xb, 1024), computed in
# chunks of 256..512 columns (fp32r needs free-dim >= 256 for full rate).
# band 7 computes cols [768,1024) but only casts/writes [896,1024).
BAND_CHUNKS = {
    0: [(0, 512), (512, 1024)],
    1: [(128, 512), (512, 1024)],
    2: [(256, 512), (512, 1024)],
    3: [(384, 512), (512, 1024)],
    4: [(512, 1024)],
    5: [(640, 1024)],
    6: [(768, 1024)],
    7: [(768, 1024)],
}

_STATE = {}


def _tables():
    """Host-precomputed constants (mimic reference fp32 ops)."""
    position = np.arange(S, dtype=np.float32)
    idx = np.arange(H // 2, dtype=np.float32)
    expo = (np.float32(-2.0) * idx) / np.float32(H)
    inv_freq = np.power(np.float32(10000.0), expo).astype(np.float32)
    ang = position[:, None] * inv_freq[None, :]          # [S, 32] f32
    cos_h = np.cos(ang).astype(np.float32).T             # [32, S]
    sin_h = np.sin(ang).astype(np.float32).T
    cos2 = np.tile(cos_h, (4, 1))                        # [128, S]
    sin2 = np.tile(sin_h, (4, 1))
    cs2 = np.ascontiguousarray(
        np.stack([cos2, sin2], axis=1).astype(np.float16))  # [128, 2, S]
    # selector lhsT [128, 512]: 4 column blocks of 128, each mapping the
    # stacked [start;end] projection rows to DUPLICATED outputs (rows 0-63
    # and 64-127 identical). msw: out[2m] = -in[2m+1]; out[2m+1] = in[2m].
    sel = np.zeros((2 * H, 4 * 2 * H), np.float32)
    for d in range(2):  # duplicate halves of the output
        mo = 64 * d
        for j in range(H):
            sel[j, 0 + mo + j] = 1.0               # start dup
            sel[H + j, 256 + mo + j] = 1.0         # end dup
        for m in range(H // 2):
            sel[2 * m + 1, 128 + mo + 2 * m] = -1.0      # start swap
            sel[2 * m, 128 + mo + 2 * m + 1] = 1.0
            sel[H + 2 * m + 1, 384 + mo + 2 * m] = -1.0  # end swap
            sel[H + 2 * m, 384 + mo + 2 * m + 1] = 1.0
    return cs2, sel


def _build():
    import concourse.bacc as bacc
    import concourse.bass as bass
    import concourse.mybir as mybir
    from concourse import tile

    f32 = mybir.dt.float32
    f32r = mybir.dt.float32r
    bf16 = mybir.dt.bfloat16
    AF = mybir.ActivationFunctionType
    ALU = mybir.AluOpType
    PSUM = bass.MemorySpace.PSUM

    nc = bacc.Bacc("TRN2", target_bir_lowering=False, debug=False,
                   num_devices=NCORES)

    f16 = mybir.dt.float16

    # xTp / wbp are host-preswizzled to [partition, ...] so every input DMA
    # lands with one large contiguous descriptor per partition; x and the
    # rotation tables ship as fp16 (2^-11 rounding, negligible next to the
    # fp32r matmul error) to halve the critical-path input bytes
    xT_t = nc.dram_tensor("xTp", [128, KT, S], f16, kind="ExternalInput")
    wb_t = nc.dram_tensor("wbp", [128, KT, 2 * H], f32r,
                          kind="ExternalInput")
    b2_t = nc.dram_tensor("bias2", [2 * H, 1], f32, kind="ExternalInput")
    wo2_t = nc.dram_tensor("wo2", [2 * H, 2, 2 * H], f32r,
                           kind="ExternalInput")
    cs_t = nc.dram_tensor("cs2", [2 * H, 2, S], f16, kind="ExternalInput")
    sel_t = nc.dram_tensor("sel4", [2 * H, 4 * 2 * H], f32r,
                           kind="ExternalInput")
    out_t = nc.dram_tensor("out", [OH, S, S], bf16, kind="ExternalOutput")

    # [o, xb, p, y]: row = 128*xb + p
    out_b = out_t.ap().rearrange("o (xb p) y -> o xb p y", xb=8, p=128)
    xg = xT_t.ap()

    def r(ap):
        return ap.bitcast(f32r)

    with tile.TileContext(nc) as tc:
        with tc.tile_pool(name="persist", bufs=1) as pp, \
             tc.tile_pool(name="scratch", bufs=2) as sp, \
             tc.tile_pool(name="stage", bufs=8) as stp, \
             tc.tile_pool(name="psu", bufs=1, space=PSUM) as psu:

            # deps are tracked per tile, so anything produced per column
            # half gets its own tile: readers of one half must not wait on
            # writers of the other
            xF1a = pp.tile([128, 3, 512], f16)
            xF1b = pp.tile([128, 3, 512], f16)
            xF0 = pp.tile([128, KT, 512], f16)
            xTr1a = pp.tile([128, 3, 512], f32r)
            xTr1b = pp.tile([128, 3, 512], f32r)
            xTr0 = pp.tile([128, KT, 512], f32r)
            wbT = pp.tile([128, KT, 2 * H], f32r)
            sel4 = pp.tile([2 * H, 4 * 2 * H], f32r)
            wo2 = pp.tile([2 * H, 2, 2 * H], f32r)
            bias2 = pp.tile([2 * H, 1], f32)
            cs2 = pp.tile([2 * H, 2, S], f16)
            startRh = [pp.tile([2 * H, 512], f32, name=f"startR{h}")
                       for h in range(2)]
            endRh = [pp.tile([2 * H, 512], f32, name=f"endR{h}")
                     for h in range(2)]
            tmp2 = pp.tile([2 * H, 4, S], f32)
            wdum = pp.tile([128, 512], f32)

            sl1 = slice(512, 1024)
            sl0 = slice(0, 512)

            # two PSUM rings of [128, 512] (1 bank) x 4 bufs = all 8
            # banks; ring depth 4 lets the PE run several chunks ahead of
            # the casting engines so semaphore latency stays off the
            # critical path
            def psa():
                return psu.tile([128, 512], f32, name="psA", tag="a",
                                bufs=4)

            def psb():
                return psu.tile([128, 512], f32, name="psB", tag="b",
                                bufs=4)

            # all inputs on the sync queue in critical-path order (one
            # FIFO = explicit priority); f16 x chunks are upcast to f32r
            # on DVE (h=1) / ACT (h=0) while later inputs stream
            nc.sync.dma_start(wbT[:], wb_t.ap())
            nc.sync.dma_start(xF1a[:], xg[:, 0:3, sl1])
            nc.sync.dma_start(xF1b[:], xg[:, 3:6, sl1])
            nc.sync.dma_start(bias2[:], b2_t.ap())
            nc.sync.dma_start(sel4[:], sel_t.ap())
            nc.sync.dma_start(cs2[:], cs_t.ap())
            nc.sync.dma_start(xF0[:], xg[:, :, sl0])
            nc.sync.dma_start(wo2[:], wo2_t.ap())
            nc.gpsimd.memset(wdum[:], 0.0)
            for t in range(3):
                nc.vector.tensor_copy(xTr1a[:, t, :], xF1a[:, t, :])
            for t in range(3):
                nc.vector.tensor_copy(xTr1b[:, t, :], xF1b[:, t, :])
            # PE warm-up: dummy matmuls during the input load so the HAM
            # window is at full clock when real work arrives
            for i in range(4):
                ps_w = psa() if i % 2 == 0 else psb()
                nc.tensor.matmul(ps_w[:], r(wdum[:, 0:128]),
                                 r(wdum[:]), start=True, stop=True)

            relu_of = {}

            def proj_h(h):
                ps2 = psa()
                if h == 1:
                    for kb in range(3):
                        nc.tensor.matmul(ps2[:], wbT[:, kb, :],
                                         xTr1a[:, kb, :],
                                         start=(kb == 0), stop=False)
                    for kb in range(3):
                        nc.tensor.matmul(ps2[:], wbT[:, kb + 3, :],
                                         xTr1b[:, kb, :],
                                         start=False, stop=(kb == 2))
                else:
                    for kb in range(KT):
                        nc.tensor.matmul(ps2[:], wbT[:, kb, :],
                                         xTr0[:, kb, :],
                                         start=(kb == 0),
                                         stop=(kb == KT - 1))
                relu2 = sp.tile([128, 512], f32, name="relu2")
                nc.scalar.activation(r(relu2[:]), ps2[:], AF.Relu,
                                     bias=bias2[:])
                relu_of[h] = relu2

            def rot_h(h, side):
                # side 0 = start rotation, 1 = end rotation; the adds run
                # on gpsimd for h=1 (its queue is still empty then) and on
                # DVE for h=0 (gpsimd is busy generating stB descriptors)
                sl = slice(h * 512, (h + 1) * 512)
                relu2 = relu_of[h]
                q = 256 * side
                ps_d = psa()
                nc.tensor.matmul(ps_d[:], sel4[:, q:q + 128],
                                 r(relu2[:]), start=True, stop=True)
                ps_w = psb()
                nc.tensor.matmul(ps_w[:], sel4[:, q + 128:q + 256],
                                 r(relu2[:]), start=True, stop=True)
                t_d = sp.tile([128, 512], f32, name="t_d")
                nc.vector.tensor_mul(t_d[:], ps_d[:], cs2[:, 0, sl])
                t_w = sp.tile([128, 512], f32, name="t_w")
                nc.vector.tensor_mul(t_w[:], ps_w[:], cs2[:, 1, sl])
                dst = (startRh if side == 0 else endRh)[h]
                nc.gpsimd.tensor_tensor(r(dst[:]), t_d[:], t_w[:], ALU.add)

            def prep_h(h):
                proj_h(h)
                rot_h(h, 0)
                rot_h(h, 1)

            def tmp_g(g, h):
                # tmp for labels (2g, 2g+1) on rows 0-63 / 64-127; groups
                # with ph=0/1 use distinct PE row groups -> concurrent
                pg, ph = g // 2, g % 2
                sl = slice(h * 512, (h + 1) * 512)
                ps_t = psa() if ph == 0 else psb()
                nc.tensor.matmul(ps_t[:],
                                 wo2[64 * ph:64 * ph + 64, pg, :],
                                 r(startRh[h][64 * ph:64 * ph + 64, :]),
                                 start=True, stop=True,
                                 tile_position=(64 * ph, 0))
                if ph == 0:
                    nc.vector.tensor_copy(r(tmp2[:, g, sl]), ps_t[:])
                else:
                    nc.scalar.copy(r(tmp2[:, g, sl]), ps_t[:])

            def band(g, xb):
                w0 = 128 * xb
                wb = 1024 - w0
                stA = stp.tile([128, 1024], bf16, name="stA")
                stB = stp.tile([128, 1024], bf16, name="stB")
                chunks = BAND_CHUNKS[xb]
                lhsA = r(tmp2[0:64, g, w0:w0 + 128])
                lhsB = r(tmp2[64:128, g, w0:w0 + 128])
                for (c0, c1) in chunks:
                    n = c1 - c0
                    eh = endRh[1] if c0 >= 512 else endRh[0]
                    e0 = c0 - 512 if c0 >= 512 else c0
                    psA = psa()
                    psB = psb()
                    nc.tensor.matmul(psA[:, 0:n], lhsA,
                                     r(eh[0:64, e0:e0 + n]),
                                     start=True, stop=True,
                                     tile_position=(0, 0))
                    nc.tensor.matmul(psB[:, 0:n], lhsB,
                                     r(eh[64:128, e0:e0 + n]),
                                     start=True, stop=True,
                                     tile_position=(64, 0))
                    d0 = max(c0, w0) - w0
                    s0 = max(0, w0 - c0)
                    nc.vector.tensor_copy(stA[:, d0:c1 - w0], psA[:, s0:n])
                    nc.scalar.copy(stB[:, d0:c1 - w0], psB[:, s0:n])
                nc.sync.dma_start(out_b[2 * g, xb][:, w0:1024],
                                  stA[:, 0:wb])
                nc.gpsimd.dma_start(out_b[2 * g + 1, xb][:, w0:1024],
                                    stB[:, 0:wb])

            # both preps first in PE program order (prep_h(0) fills the PE
            # gap while h=1 rotation runs on DVE/gpsimd), then bands 4-7
            # (h=1 data only), then the h=0 tmp and remaining bands; g
            # varies fastest so consecutive per-engine instructions are
            # independent and semaphore latencies overlap
            prep_h(1)
            # h=0 x upcasts on ACT, after relu1 in its queue order
            for t in range(KT):
                nc.scalar.copy(xTr0[:, t, :], xF0[:, t, :])
            # bands 4-7 need only h=1 data; prep_h(0)'s stages are woven
            # between them so its serial chain (proj -> relu -> selector
            # -> rotate) overlaps the band casts and tmp0 is ready the
            # moment band group 0 comes up
            for g in range(4):
                tmp_g(g, 1)
            for g in range(4):
                band(g, 4)
            prep_h(0)
            for xb in (5, 6, 7):
                for g in range(4):
                    band(g, xb)
            for g in range(4):
                tmp_g(g, 0)
            for xb in (0, 1, 2, 3):
                for g in range(4):
                    band(g, xb)

    nc.compile()
    return nc


def _get_nc():
    if "nc" not in _STATE:
        _STATE["nc"] = _build()
    return _STATE["nc"]


def _make_in_maps(x, mask, W_start, b_start, W_end, b_end, weight):
    cs2, sel = _tables()
    x = np.asarray(x, np.float32)
    W_start = np.asarray(W_start, np.float32)
    W_end = np.asarray(W_end, np.float32)
    w_both = np.ascontiguousarray(np.concatenate([W_start, W_end], axis=1))
    bias2 = np.ascontiguousarray(
        np.concatenate([np.asarray(b_start, np.float32).reshape(H),
                        np.asarray(b_end, np.float32).reshape(H)]).reshape(
                            2 * H, 1))
    weight = np.asarray(weight, np.float32)
    # pre-swizzle to [partition, t, ...] so DMA descriptors are one large
    # contiguous run per partition; ship x as fp16
    xTs = [np.ascontiguousarray(
        x[b].T.reshape(KT, 128, S).transpose(1, 0, 2).astype(np.float16))
        for b in range(B)]
    wbp = np.ascontiguousarray(
        w_both.reshape(KT, 128, 2 * H).transpose(1, 0, 2))
    wo2s = []
    for half in range(2):
        wl = weight[half * OH:(half + 1) * OH]
        wo2 = np.zeros((2 * H, 2, 2 * H), np.float32)
        for pg in range(2):
            for ph in range(2):
                for u in range(2):
                    o = 2 * (2 * pg + ph) + u
                    wo2[64 * ph:64 * ph + 64, pg, 64 * u:64 * u + 64] = wl[o]
        wo2s.append(np.ascontiguousarray(wo2))
    in_maps = []
    for c in range(NCORES):
        b, half = c // 2, c % 2
        in_maps.append({
            "xTp": xTs[b],
            "wbp": wbp,
            "bias2": bias2,
            "wo2": wo2s[half],
            "cs2": cs2,
            "sel4": sel,
        })
    return in_maps


def _assemble(outs, mask):
    """Gather per-core band outputs into the full fp32 result, filling the
    mask-determined entries (masked columns, below-diagonal region) with
    their exact fp32 values."""
    mask = np.asarray(mask, np.float32)
    full = np.empty((B, O, S, S), np.float32)
    for c in range(NCORES):
        b, half = c // 2, c % 2
        full[b, half * OH:(half + 1) * OH] = \
            np.asarray(outs[c]).astype(np.float32)
    tri = np.tri(S, S, -1, dtype=bool)  # [x, y]: x > y
    for b in range(B):
        pad = mask[b]
        cols0 = np.nonzero(pad == 0.0)[0]
        if cols0.size:
            full[b][:, :, cols0] = np.float32(-NEG)
        below = (np.float32(-NEG) * (np.float32(2.0) - pad)).astype(
            np.float32)                                   # [y]
        full[b][:, tri] = np.broadcast_to(below, (S, S))[tri]
    return full


def _execute(in_maps, trace=False):
    from concourse.bass_utils import run_bass_kernel_spmd
    nc = _get_nc()
    return run_bass_kernel_spmd(nc, in_maps, list(range(NCORES)), trace=trace)


def kernel(x, mask, W_start, b_start, W_end, b_end, weight):
    in_maps = _make_in_maps(x, mask, W_start, b_start, W_end, b_end, weight)
    res = _execute(in_maps)
    outs = [res.results[c]["out"] for c in range(NCORES)]
    return _assemble(outs, mask)
